# revision 1
# baseline (speedup 1.0000x reference)
"""CoNHD GD-layer Trainium2 kernel (8-core SPMD, Bass/Tile).

Math (see module docstring of the reference): two independent set-attention
stacks over fixed-size mailbox groups (v-side: N=2048 nodes x DV=32, e-side:
M=4096 hyperedges x DE=16), followed by a 4*D -> D update linear applied in
two eid orders.

Device strategy:
  - Shard rows (E=65536) across 8 cores: core c owns v-rows and e-rows
    [c*8192, (c+1)*8192).  Group attention never crosses that boundary.
  - Transposed activation layout on chip: SBUF tiles are [feat, rows].
  - All matmuls in fp32r (full PE rate at moving-dim 256, ~1e-4 rel err).
  - Block-diagonal group masking is folded into the score matmul as a
    rank-G accumulation (Gk^T @ Gq, scaled 16C), removed by exp(x/16 - C).
  - Softmax denominator comes from a ones-column appended to V (65-stride
    layout); normalization uses a K=1 ones-matmul broadcast of 1/denom.
  - The update linear is decomposed by column blocks of upd_W:
      A  = co_in@W1 + co_v@W2 + co_0@W4 + b   ('in' eid order, on device)
      P3 = co_e@W3                            ('con' eid order, on device)
      out_in  = A + P3[inv_perm]              (host add + gather)
      out_con = out_in[perm]                  (host gather)

kernel(**inputs) takes the full unsharded inputs and returns [2, E, D] f32.
"""
import sys

if "/opt/trn_rl_repo" not in sys.path:
    sys.path.insert(0, "/opt/trn_rl_repo")

from contextlib import ExitStack

import numpy as np

import concourse.mybir as mybir
import concourse.tile as tile
from concourse import bacc
from concourse.bass_utils import run_bass_kernel_spmd

F32 = mybir.dt.float32
F32R = mybir.dt.float32r
AF = mybir.ActivationFunctionType

N, DV, M, DE, E = 2048, 32, 4096, 16, 65536
D, WD, L, H = 256, 64, 2, 4
NCORES = 8
MASK_C = 30.0


def _sab_tile(nc, pools, Xt, W, Bcol, bvbc, Gk, Gq, ones1, negc):
    """One SAB layer on one 256-row tile (Xt = [feat,rows] tile pair)."""
    sb, psum_mm, psum_S, psum_O = pools

    Qt = [sb.tile([128, 256], F32R, tag="Qt", name="Qt") for _ in range(2)]
    Kt = [sb.tile([128, 256], F32R, tag="Kt", name="Kt") for _ in range(2)]
    for fb in range(2):
        psQ = psum_mm.tile([128, 256], F32, tag="mm", name="psQ")
        for kb in range(2):
            nc.tensor.matmul(psQ[:], W["q"][kb][:, fb * 128:(fb + 1) * 128],
                             Xt[kb][:], start=(kb == 0), stop=(kb == 1))
        nc.vector.tensor_scalar_add(Qt[fb][:], psQ[:], Bcol[:, 0 * 2 + fb:0 * 2 + fb + 1])
        psK = psum_mm.tile([128, 256], F32, tag="mm", name="psK")
        for kb in range(2):
            nc.tensor.matmul(psK[:], W["k"][kb][:, fb * 128:(fb + 1) * 128],
                             Xt[kb][:], start=(kb == 0), stop=(kb == 1))
        nc.vector.tensor_scalar_add(Kt[fb][:], psK[:], Bcol[:, 1 * 2 + fb:1 * 2 + fb + 1])

    # V in row-major 65-stride layout; col h*65+64 holds ones -> softmax denom
    V65 = []
    for rb in range(2):
        psV = psum_mm.tile([128, 256], F32, tag="mm", name="psV")
        for kb in range(2):
            nc.tensor.matmul(psV[:], Xt[kb][:, rb * 128:(rb + 1) * 128],
                             W["v"][kb][:], start=(kb == 0), stop=(kb == 1))
        v65 = sb.tile([128, 4 * 65], F32R, tag="V65", name="V65")
        for h in range(H):
            nc.vector.tensor_add(v65[:, h * 65:h * 65 + 64],
                                 psV[:, h * 64:(h + 1) * 64],
                                 bvbc[:, h * 64:(h + 1) * 64])
        nc.vector.tensor_copy(v65[:, 64::65], ones1[:, 0:4])
        V65.append(v65)

    # attention per head-pair (2 psO banks live at a time)
    Ot = [sb.tile([128, 256], F32R, tag="Ot", name="Ot") for _ in range(2)]
    for pair in range(2):
        psO, recips = [], []
        for hh in range(2):
            h = pair * 2 + hh
            off = hh * 64
            Qht = Qt[pair][off:off + 64, :]
            Kht = Kt[pair][off:off + 64, :]
            eS = []
            for b in range(2):
                psS = psum_S.tile([128, 256], F32, tag="psS", name="psS")
                nc.tensor.matmul(psS[:], Kht[:, b * 128:(b + 1) * 128], Qht,
                                 start=True, stop=False)
                nc.tensor.matmul(psS[:], Gk[:], Gq[b][:], start=False, stop=True)
                e = sb.tile([128, 256], F32R, tag="eS", name="eS")
                nc.scalar.activation(e[:], psS[:], AF.Exp, bias=negc[:], scale=1.0 / 16.0)
                eS.append(e)
            pO = psum_O.tile([65, 256], F32, tag="psO", name="psO")
            for b in range(2):
                nc.tensor.matmul(pO[:], V65[b][:, h * 65:h * 65 + 65], eS[b][:],
                                 start=(b == 0), stop=(b == 1))
            rec = sb.tile([1, 256], F32R, tag="recipH", name="recipH")
            nc.vector.reciprocal(rec[:], pO[64:65, :])
            psO.append(pO)
            recips.append(rec)
        RB = sb.tile([128, 256], F32, tag="RB", name="RB")
        for hh in range(2):
            psRB = psum_S.tile([64, 256], F32, tag="psS", name="psRB")
            nc.tensor.matmul(psRB[:], ones1[0:1, 0:64], recips[hh][:],
                             start=True, stop=True)
            nc.scalar.copy(RB[hh * 64:(hh + 1) * 64, :], psRB[:])
        for hh in range(2):
            off = hh * 64
            nc.vector.tensor_mul(Ot[pair][off:off + 64, :], psO[hh][0:64, :],
                                 RB[off:off + 64, :])
            nc.vector.tensor_add(Ot[pair][off:off + 64, :], Ot[pair][off:off + 64, :],
                                 Qt[pair][off:off + 64, :])

    # Z = O + relu(O @ Wo + bo)
    Zt = [sb.tile([128, 256], F32R, tag="Zt", name="Zt") for _ in range(2)]
    for fb in range(2):
        psR = psum_mm.tile([128, 256], F32, tag="mm", name="psR")
        for kb in range(2):
            nc.tensor.matmul(psR[:], W["o"][kb][:, fb * 128:(fb + 1) * 128],
                             Ot[kb][:], start=(kb == 0), stop=(kb == 1))
        Rt = sb.tile([128, 256], F32, tag="Rt", name="Rt")
        nc.scalar.activation(Rt[:], psR[:], AF.Relu,
                             bias=Bcol[:, 3 * 2 + fb:3 * 2 + fb + 1])
        nc.vector.tensor_add(Zt[fb][:], Ot[fb][:], Rt[:])
    return Zt


def _load_side_consts(nc, const, tag, W_d, Bcol_d, bvbc_d, Gk_d, Gq_d, G):
    Ws, Bcols, bvbcs = [], [], []
    for l in range(L):
        Wl = {}
        for pi, p in enumerate(["q", "k", "v", "o"]):
            Wl[p] = []
            for kb in range(2):
                t = const.tile([128, 256], F32R, tag=f"{tag}W{l}{p}{kb}",
                               name=f"{tag}W{l}{p}{kb}")
                nc.sync.dma_start(t[:], W_d[l, pi, kb * 128:(kb + 1) * 128, :])
                Wl[p].append(t)
        Ws.append(Wl)
        bc = const.tile([128, 8], F32, tag=f"{tag}Bcol{l}", name=f"{tag}Bcol{l}")
        nc.sync.dma_start(bc[:], Bcol_d[l])
        Bcols.append(bc)
        bv = const.tile([128, 256], F32, tag=f"{tag}bvbc{l}", name=f"{tag}bvbc{l}")
        nc.sync.dma_start(bv[:], bvbc_d[l])
        bvbcs.append(bv)
    Gk = const.tile([G, 128], F32R, tag=f"{tag}Gk", name=f"{tag}Gk")
    nc.sync.dma_start(Gk[:], Gk_d)
    Gq = []
    for b in range(2):
        g = const.tile([G, 256], F32R, tag=f"{tag}Gq{b}", name=f"{tag}Gq{b}")
        nc.sync.dma_start(g[:], Gq_d[b])
        Gq.append(g)
    return Ws, Bcols, bvbcs, Gk, Gq


def build_program(R):
    """Build the per-core SPMD program; R = rows per core (multiple of 256)."""
    NT = R // 256
    nc = bacc.Bacc("TRN2", target_bir_lowering=False, debug=False)

    dram = {}

    def din(name, shape, dt=F32R):
        dram[name] = nc.dram_tensor(name, shape, dt, kind="ExternalInput").ap()
        return dram[name]

    xvt_d = din("xvt", [D, R])
    wvt_d = din("wvt", [WD, R])
    xet_d = din("xet", [D, R])
    wet_d = din("wet", [WD, R])
    x0t_d = din("x0t", [D, R])
    peW_v_d = din("peW_v", [WD, D])
    peW_e_d = din("peW_e", [WD, D])
    peb_v_d = din("peb_v", [D], F32)
    peb_e_d = din("peb_e", [D], F32)
    Wv_d = din("W_v", [L, 4, D, D])
    We_d = din("W_e", [L, 4, D, D])
    Bcol_v_d = din("Bcol_v", [L, 128, 8], F32)
    Bcol_e_d = din("Bcol_e", [L, 128, 8], F32)
    bvbc_v_d = din("bvbc_v", [L, 128, D], F32)
    bvbc_e_d = din("bvbc_e", [L, 128, D], F32)
    Wupd_d = din("W_upd", [4, D, D])
    updb_d = din("updb_bc", [128, D], F32)
    Gk_v_d = din("Gk_v", [4, 128])
    Gq_v_d = din("Gq_v", [2, 4, 256])
    Gk_e_d = din("Gk_e", [8, 128])
    Gq_e_d = din("Gq_e", [2, 8, 256])
    ones1_d = din("ones1", [128, 128])

    A_d = nc.dram_tensor("A", [R, D], F32, kind="ExternalOutput").ap()
    P3_d = nc.dram_tensor("P3", [R, D], F32, kind="ExternalOutput").ap()

    with tile.TileContext(nc) as tc, ExitStack() as es, \
            nc.allow_low_precision(reason="fp32r matmul pipeline, fp32 accum in PSUM"):
        const = es.enter_context(tc.tile_pool(name="const", bufs=1))
        sb = es.enter_context(tc.tile_pool(name="sb", bufs=4))
        inp = es.enter_context(tc.tile_pool(name="inp", bufs=4))
        outp = es.enter_context(tc.tile_pool(name="outp", bufs=4))
        psum_mm = es.enter_context(tc.tile_pool(name="psmm", bufs=3, space="PSUM"))
        psum_S = es.enter_context(tc.tile_pool(name="psS", bufs=3, space="PSUM"))
        psum_O = es.enter_context(tc.tile_pool(name="psO", bufs=2, space="PSUM"))
        pools = (sb, psum_mm, psum_S, psum_O)

        negc = const.tile([128, 1], F32, tag="negc", name="negc")
        nc.vector.memset(negc[:], -MASK_C)
        ones1 = const.tile([128, 128], F32R, tag="ones1", name="ones1")
        nc.sync.dma_start(ones1[:], ones1_d)

        peW = {}
        peb = {}
        for s, peW_d, peb_d in (("v", peW_v_d, peb_v_d), ("e", peW_e_d, peb_e_d)):
            t = const.tile([WD, D], F32R, tag=f"peW_{s}", name=f"peW_{s}")
            nc.sync.dma_start(t[:], peW_d)
            peW[s] = t
            b = const.tile([128, 2], F32, tag=f"peb_{s}", name=f"peb_{s}")
            for fb in range(2):
                nc.sync.dma_start(b[:, fb:fb + 1],
                                  peb_d[fb * 128:(fb + 1) * 128].unsqueeze(-1))
            peb[s] = b

        side_consts = {
            "v": _load_side_consts(nc, const, "v", Wv_d, Bcol_v_d, bvbc_v_d,
                                   Gk_v_d, Gq_v_d, 4),
            "e": _load_side_consts(nc, const, "e", We_d, Bcol_e_d, bvbc_e_d,
                                   Gk_e_d, Gq_e_d, 8),
        }

        Wupd = []
        for j in range(4):
            Wupd.append([])
            for kb in range(2):
                t = const.tile([128, 256], F32R, tag=f"Wupd{j}{kb}", name=f"Wupd{j}{kb}")
                nc.sync.dma_start(t[:], Wupd_d[j, kb * 128:(kb + 1) * 128, :])
                Wupd[j].append(t)
        updb = const.tile([128, 256], F32, tag="updb", name="updb")
        nc.sync.dma_start(updb[:], updb_d)

        for side in ("v", "e"):
            Ws, Bcols, bvbcs, Gk, Gq = side_consts[side]
            xt_d, wt_d = (xvt_d, wvt_d) if side == "v" else (xet_d, wet_d)
            for t in range(NT):
                cs = slice(t * 256, (t + 1) * 256)
                xt = [inp.tile([128, 256], F32R, tag=f"xt{side}", name="xt")
                      for _ in range(2)]
                for fb in range(2):
                    nc.sync.dma_start(xt[fb][:], xt_d[fb * 128:(fb + 1) * 128, cs])
                wt = inp.tile([WD, 256], F32R, tag=f"wt{side}", name="wt")
                nc.sync.dma_start(wt[:], wt_d[:, cs])

                # mailbox: Xt = xt + peW^T wt + peb
                Xt = [sb.tile([128, 256], F32R, tag="Xt", name="Xt") for _ in range(2)]
                for fb in range(2):
                    psP = psum_mm.tile([128, 256], F32, tag="mm", name="psP")
                    nc.tensor.matmul(psP[:], peW[side][:, fb * 128:(fb + 1) * 128],
                                     wt[:], start=True, stop=True)
                    nc.vector.scalar_tensor_tensor(
                        Xt[fb][:], psP[:], peb[side][:, fb:fb + 1], xt[fb][:],
                        mybir.AluOpType.add, mybir.AluOpType.add)

                for l in range(L):
                    Xt = _sab_tile(nc, pools, Xt, Ws[l], Bcols[l], bvbcs[l],
                                   Gk, Gq, ones1, negc)

                if side == "v":
                    x0 = [inp.tile([128, 256], F32R, tag="x0", name="x0")
                          for _ in range(2)]
                    for fb in range(2):
                        nc.sync.dma_start(x0[fb][:], x0t_d[fb * 128:(fb + 1) * 128, cs])
                    for rb in range(2):
                        rs = slice(rb * 128, (rb + 1) * 128)
                        psA = psum_mm.tile([128, 256], F32, tag="mm", name="psA")
                        first = True
                        for src, j in ((xt, 0), (Xt, 1), (x0, 3)):
                            for kb in range(2):
                                nc.tensor.matmul(psA[:], src[kb][:, rs], Wupd[j][kb][:],
                                                 start=first,
                                                 stop=(src is x0 and kb == 1))
                                first = False
                        Ao = outp.tile([128, 256], F32, tag="Aout", name="Aout")
                        nc.vector.tensor_add(Ao[:], psA[:], updb[:])
                        nc.sync.dma_start(A_d[t * 256 + rb * 128:t * 256 + (rb + 1) * 128, :],
                                          Ao[:])
                else:
                    for rb in range(2):
                        rs = slice(rb * 128, (rb + 1) * 128)
                        psP3 = psum_mm.tile([128, 256], F32, tag="mm", name="psP3")
                        for kb in range(2):
                            nc.tensor.matmul(psP3[:], Xt[kb][:, rs], Wupd[2][kb][:],
                                             start=(kb == 0), stop=(kb == 1))
                        Po = outp.tile([128, 256], F32, tag="Pout", name="Pout")
                        nc.vector.tensor_copy(Po[:], psP3[:])
                        nc.sync.dma_start(P3_d[t * 256 + rb * 128:t * 256 + (rb + 1) * 128, :],
                                          Po[:])

    nc.compile()
    return nc


def _make_group_consts(n_group):
    G = 128 // n_group
    Gk = np.zeros((G, 128), np.float32)
    for g in range(G):
        Gk[g, g * n_group:(g + 1) * n_group] = 16.0 * MASK_C
    Gq = np.zeros((2, G, 256), np.float32)
    for b in range(2):
        for g in range(G):
            q0 = b * 128 + g * n_group
            Gq[b, g, q0:q0 + n_group] = 1.0
    return Gk, Gq


def _pack_bcol(bq, bk, bv, bo):
    """[128, 8] bias columns per layer: col p*2+fb."""
    out = np.zeros((L, 128, 8), np.float32)
    for l in range(L):
        for pi, b in enumerate((bq, bk, bv, bo)):
            for fb in range(2):
                out[l, :, pi * 2 + fb] = b[l, fb * 128:(fb + 1) * 128]
    return out


_PROGRAM_CACHE = {}


def _get_program(R):
    if R not in _PROGRAM_CACHE:
        _PROGRAM_CACHE[R] = build_program(R)
    return _PROGRAM_CACHE[R]


def kernel(co_feat_in, co_feat_con, co_feat_0, weight_in, weight_con,
           pe_v_W, pe_v_b, pe_e_W, pe_e_b,
           Wq_v, bq_v, Wk_v, bk_v, Wv_v, bv_v, Wo_v, bo_v,
           Wq_e, bq_e, Wk_e, bk_e, Wv_e, bv_e, Wo_e, bo_e,
           upd_W, upd_b, perm):
    f = np.asarray
    co_feat_in = f(co_feat_in, np.float32)
    co_feat_con = f(co_feat_con, np.float32)
    co_feat_0 = f(co_feat_0, np.float32)
    weight_in = f(weight_in, np.float32)
    weight_con = f(weight_con, np.float32)
    perm = np.asarray(perm)

    R = E // NCORES
    nc = _get_program(R)

    Gk_v, Gq_v = _make_group_consts(DV)
    Gk_e, Gq_e = _make_group_consts(DE)

    shared = {
        "peW_v": f(pe_v_W, np.float32), "peW_e": f(pe_e_W, np.float32),
        "peb_v": f(pe_v_b, np.float32), "peb_e": f(pe_e_b, np.float32),
        "W_v": np.stack([f(Wq_v, np.float32), f(Wk_v, np.float32),
                         f(Wv_v, np.float32), f(Wo_v, np.float32)], axis=1).copy(),
        "W_e": np.stack([f(Wq_e, np.float32), f(Wk_e, np.float32),
                         f(Wv_e, np.float32), f(Wo_e, np.float32)], axis=1).copy(),
        "Bcol_v": _pack_bcol(f(bq_v, np.float32), f(bk_v, np.float32),
                             f(bv_v, np.float32), f(bo_v, np.float32)),
        "Bcol_e": _pack_bcol(f(bq_e, np.float32), f(bk_e, np.float32),
                             f(bv_e, np.float32), f(bo_e, np.float32)),
        "bvbc_v": np.ascontiguousarray(
            np.broadcast_to(f(bv_v, np.float32)[:, None, :], (L, 128, D))),
        "bvbc_e": np.ascontiguousarray(
            np.broadcast_to(f(bv_e, np.float32)[:, None, :], (L, 128, D))),
        "W_upd": np.ascontiguousarray(
            f(upd_W, np.float32).reshape(4, D, D)),
        "updb_bc": np.ascontiguousarray(
            np.broadcast_to(f(upd_b, np.float32)[None, :], (128, D))),
        "Gk_v": Gk_v, "Gq_v": Gq_v, "Gk_e": Gk_e, "Gq_e": Gq_e,
        "ones1": np.ones((128, 128), np.float32),
    }

    in_maps = []
    for c in range(NCORES):
        rs = slice(c * R, (c + 1) * R)
        m = dict(shared)
        m["xvt"] = np.ascontiguousarray(co_feat_in[rs].T)
        m["wvt"] = np.ascontiguousarray(weight_in[rs].T)
        m["xet"] = np.ascontiguousarray(co_feat_con[rs].T)
        m["wet"] = np.ascontiguousarray(weight_con[rs].T)
        m["x0t"] = np.ascontiguousarray(co_feat_0[rs].T)
        in_maps.append(m)

    global _last_in_maps
    _last_in_maps = in_maps
    res = run_bass_kernel_spmd(nc, in_maps, core_ids=list(range(NCORES)))
    A = np.concatenate([res.results[c]["A"] for c in range(NCORES)], axis=0)
    P3 = np.concatenate([res.results[c]["P3"] for c in range(NCORES)], axis=0)

    inv_perm = np.argsort(perm)
    out_in = A + P3[inv_perm]
    return np.stack([out_in, out_in[perm]]).astype(np.float32)



# revision 38
# speedup vs baseline: 1.2410x; 1.2410x over previous
"""CoNHD GD-layer Trainium2 kernel (8-core SPMD, Bass/Tile).

Math (see the reference): two independent set-attention stacks over
fixed-size mailbox groups (v-side: N=2048 nodes x DV=32, e-side: M=4096
hyperedges x DE=16), followed by a 4*D -> D update linear applied in two
eid orders.

Device strategy (v2 — bf16 pipeline, engine-balanced):
  - Shard rows (E=65536) across 8 cores: core c owns v-rows and e-rows
    [c*8192, (c+1)*8192).  Group attention never crosses that boundary.
  - All SBUF activations and weights are bf16; PSUM accumulates fp32.
  - PSUM is bank-granular (2KB/partition), so PSUM tiles are full-bank
    [128, 512] f32 pairs; SBUF activations use matching [128, 512]
    paired layouts (col = half*256 + r).
  - Group structure is block-diagonal at 128-row granularity (DV=32,
    DE=16 both divide 128), so scores/AV/denominator matmuls are
    restricted to aligned 128x128 blocks — half the work of a dense
    256-wide attention.
  - In-block group masking is a single rank-G matmul (Gk^T @ Gq4, scaled
    16C) per key-block covering all 4 heads; removed by exp(x/16 - C).
  - A@V is computed transposed (queries on partitions, moving dim = 64
    head features), which makes the softmax denominators per-partition:
    one cheap [128,4] reciprocal per key-block, a broadcast-AP tensor_mul
    to normalize, then a PE transpose (identity matmul) back to
    feat-major for the residual add.
  - Q bias rides the PSUM->SBUF move on DVE (TSP), K bias on Act
    (Identity+bias), V bias on the Pool move, O-proj bias on the Relu,
    update bias on the Pool output move.
  - Elementwise work is spread across DVE, Act and Pool (GPSIMD) so the
    Tensor engine is the only near-critical resource.
  - Update linear, decomposed by column blocks of upd_W:
      A  = co_in@W1 + co_v@W2 + co_0@W4 + b   ('in' eid order, on device)
      P3 = co_e@W3                            ('con' eid order, on device)
      out_in  = A + P3[inv_perm]              (host add + gather)
      out_con = out_in[perm]                  (host gather)

kernel(**inputs) takes the full unsharded inputs and returns [2, E, D] f32.
"""
import sys

if "/opt/trn_rl_repo" not in sys.path:
    sys.path.insert(0, "/opt/trn_rl_repo")

from contextlib import ExitStack

import ml_dtypes
import numpy as np

import concourse.mybir as mybir
import concourse.tile as tile
from concourse import bacc
from concourse.bass_utils import run_bass_kernel_spmd

F32 = mybir.dt.float32
BF16 = mybir.dt.bfloat16
AF = mybir.ActivationFunctionType
NPBF16 = np.dtype(ml_dtypes.bfloat16)

N, DV, M, DE, E = 2048, 32, 4096, 16, 65536
D, WD, L, H = 256, 64, 2, 4
NCORES = 8
MASK_C = 30.0


def _sab_tile(nc, pools, Xt, W, Bcol, bvbc, bvrow, Gk, Gq4, ident, ones128,
              negc, v_on_act, stage=9):
    """One SAB layer on one 256-row tile.

    Xt: [128, 512] bf16, col = kb*256 + r (feat-major halves).
    Returns Zt in the same layout.
    """
    sb, psmm, psS_pool, psO_pool, psT_pool = pools

    # Q (bias via DVE TSP), K (bias via Act Identity) — feat-major halves
    Qt = sb.tile([128, 512], BF16, tag="Qt", name="Qt")
    Kt = sb.tile([128, 512], BF16, tag="Kt", name="Kt")
    psQ = psmm.tile([128, 512], F32, tag="mm", name="psQ")
    psK = psmm.tile([128, 512], F32, tag="mm", name="psK")
    for fb in range(2):
        fs = slice(fb * 128, (fb + 1) * 128)
        cs = slice(fb * 256, (fb + 1) * 256)
        for kb in range(2):
            ks = slice(kb * 256, (kb + 1) * 256)
            nc.tensor.matmul(psQ[:, cs], W["q"][kb][:, fs], Xt[:, ks],
                             start=(kb == 0), stop=(kb == 1))
            nc.tensor.matmul(psK[:, cs], W["k"][kb][:, fs], Xt[:, ks],
                             start=(kb == 0), stop=(kb == 1))
    for fb in range(2):
        cs = slice(fb * 256, (fb + 1) * 256)
        nc.vector.tensor_scalar_add(Qt[:, cs], psQ[:, cs],
                                    Bcol[:, 0 * 2 + fb:0 * 2 + fb + 1])
        nc.scalar.activation(Kt[:, cs], psK[:, cs], AF.Identity,
                             bias=Bcol[:, 1 * 2 + fb:1 * 2 + fb + 1])

    if stage <= 1:
        return Qt

    # V row-major halves (col = rb*256 + f); move to SBUF on DVE (with the
    # bias as a broadcast tensor_add) or on Act (bias as a rank-1 matmul)
    Vt = sb.tile([128, 512], BF16, tag="Vt", name="Vt")
    psV = psmm.tile([128, 512], F32, tag="mm", name="psV")
    for rb in range(2):
        cs = slice(rb * 256, (rb + 1) * 256)
        if v_on_act:
            nc.tensor.matmul(psV[:, cs], ones128[0:1, :], bvrow[:, 0:256],
                             start=True, stop=False)
        for kb in range(2):
            nc.tensor.matmul(psV[:, cs],
                             Xt[:, kb * 256 + rb * 128:kb * 256 + (rb + 1) * 128],
                             W["v"][kb][:],
                             start=(kb == 0 and not v_on_act),
                             stop=(kb == 1))
    if v_on_act:
        nc.scalar.copy(Vt[:], psV[:])
    else:
        bvb = bvbc[:].unsqueeze(1).broadcast_to([128, 2, 256])
        nc.vector.tensor_add(Vt[:], psV[:], bvb)

    if stage <= 2:
        return Vt

    # scores: block-diagonal at 128 granularity; all 4 heads share one
    # [128, 512] PSUM tile per key-block; mask is one rank-G matmul
    eS = []
    for b in range(2):
        psS = psS_pool.tile([128, 512], F32, tag="S", name="psS")
        for h in range(H):
            fb, hh = h // 2, h % 2
            hs = slice(hh * 64, (hh + 1) * 64)
            qs = slice(fb * 256 + b * 128, fb * 256 + (b + 1) * 128)
            reg = psS[:, h * 128:(h + 1) * 128]
            nc.tensor.matmul(reg, Gk[:], Gq4[:, 0:128], start=True, stop=False)
            nc.tensor.matmul(reg, Kt[hs, qs], Qt[hs, qs],
                             start=False, stop=True)
        e = sb.tile([128, 512], BF16, tag="eS", name="eS")
        nc.scalar.activation(e[:], psS[:], AF.Exp, bias=negc[:], scale=1.0 / 16.0)
        eS.append(e)

    if stage <= 3:
        return eS[0]

    # A@V transposed (queries on partitions) + per-q denominators.
    # psOD[b]: cols 0-255 = Oq (head-major feats), cols 256-259 = denom.
    OqN = sb.tile([128, 512], BF16, tag="OqN", name="OqN")
    rec = sb.tile([128, 8], BF16, tag="rec", name="rec")
    for b in range(2):
        psOD = psO_pool.tile([128, 512], F32, tag="OD", name="psOD")
        for h in range(H):
            eSh = eS[b][:, h * 128:(h + 1) * 128]
            nc.tensor.matmul(psOD[:, h * 64:(h + 1) * 64],
                             eSh, Vt[:, b * 256 + h * 64:b * 256 + (h + 1) * 64],
                             start=True, stop=True)
            nc.tensor.matmul(psOD[:, 256 + h:257 + h], eSh, ones128[:, 0:1],
                             start=True, stop=True)
        nc.vector.reciprocal(rec[:, b * 4:(b + 1) * 4], psOD[:, 256:260])
        recB = rec[:, b * 4:(b + 1) * 4].unsqueeze(-1).broadcast_to([128, 4, 64])
        nc.vector.tensor_mul(OqN[:, b * 256:(b + 1) * 256], psOD[:, 0:256], recB)

    if stage <= 4:
        return OqN

    # PE transpose back to feat-major halves, then one residual add
    psOT = psT_pool.tile([128, 512], BF16, tag="OT", name="psOT")
    for fb in range(2):
        for b in range(2):
            nc.tensor.transpose(
                psOT[:, fb * 256 + b * 128:fb * 256 + (b + 1) * 128],
                OqN[:, b * 256 + fb * 128:b * 256 + (fb + 1) * 128], ident[:])
    Ot = sb.tile([128, 512], BF16, tag="Ot", name="Ot")
    nc.vector.tensor_add(Ot[:], psOT[:], Qt[:])

    if stage <= 5:
        return Ot

    # Z = O + relu(O @ Wo + bo); relu carries the bias, Z-add on Pool
    psZ = psmm.tile([128, 512], F32, tag="mm", name="psZ")
    for fb in range(2):
        fs = slice(fb * 128, (fb + 1) * 128)
        cs = slice(fb * 256, (fb + 1) * 256)
        for kb in range(2):
            nc.tensor.matmul(psZ[:, cs], W["o"][kb][:, fs],
                             Ot[:, kb * 256:(kb + 1) * 256],
                             start=(kb == 0), stop=(kb == 1))
    Rt = sb.tile([128, 512], BF16, tag="Rt", name="Rt")
    for fb in range(2):
        cs = slice(fb * 256, (fb + 1) * 256)
        nc.scalar.activation(Rt[:, cs], psZ[:, cs], AF.Relu,
                             bias=Bcol[:, 3 * 2 + fb:3 * 2 + fb + 1])
    Zt = sb.tile([128, 512], BF16, tag="Zt", name="Zt")
    nc.gpsimd.tensor_add(Zt[:], Ot[:], Rt[:])
    return Zt


def _load_side_consts(nc, const, tag, W_d, Bcol_d, bvbc_d, bvrow_d, Gk_d,
                      Gq4_d, G):
    Ws, Bcols, bvbcs, bvrows = [], [], [], []
    for l in range(L):
        Wl = {}
        for pi, p in enumerate(["q", "k", "v", "o"]):
            Wl[p] = []
            for kb in range(2):
                t = const.tile([128, 256], BF16, tag=f"{tag}W{l}{p}{kb}",
                               name=f"{tag}W{l}{p}{kb}")
                nc.sync.dma_start(t[:], W_d[l, pi, kb * 128:(kb + 1) * 128, :])
                Wl[p].append(t)
        Ws.append(Wl)
        bc = const.tile([128, 8], F32, tag=f"{tag}Bcol{l}", name=f"{tag}Bcol{l}")
        nc.sync.dma_start(bc[:], Bcol_d[l])
        Bcols.append(bc)
        bv = const.tile([128, 256], F32, tag=f"{tag}bvbc{l}", name=f"{tag}bvbc{l}")
        nc.sync.dma_start(bv[:], bvbc_d[l])
        bvbcs.append(bv)
        br = const.tile([1, 512], BF16, tag=f"{tag}bvrow{l}", name=f"{tag}bvrow{l}")
        nc.sync.dma_start(br[:], bvrow_d[l])
        bvrows.append(br)
    Gk = const.tile([G, 128], BF16, tag=f"{tag}Gk", name=f"{tag}Gk")
    nc.sync.dma_start(Gk[:], Gk_d)
    Gq4 = const.tile([G, 512], BF16, tag=f"{tag}Gq4", name=f"{tag}Gq4")
    nc.sync.dma_start(Gq4[:], Gq4_d)
    return Ws, Bcols, bvbcs, bvrows, Gk, Gq4


def build_program(R, stage=9):
    """Build the per-core SPMD program; R = rows per core (multiple of 256)."""
    NT = R // 256
    nc = bacc.Bacc("TRN2", target_bir_lowering=False, debug=False)

    dram = {}

    def din(name, shape, dt=BF16):
        dram[name] = nc.dram_tensor(name, shape, dt, kind="ExternalInput").ap()
        return dram[name]

    xvt_d = din("xvt", [D, R])
    wvt_d = din("wvt", [WD + 1, R])
    xet_d = din("xet", [D, R])
    wet_d = din("wet", [WD + 1, R])
    x0t_d = din("x0t", [D, R])
    peW_v_d = din("peW_v", [WD + 1, D])
    peW_e_d = din("peW_e", [WD + 1, D])
    Wv_d = din("W_v", [L, 4, D, D])
    We_d = din("W_e", [L, 4, D, D])
    Bcol_v_d = din("Bcol_v", [L, 128, 8], F32)
    Bcol_e_d = din("Bcol_e", [L, 128, 8], F32)
    bvbc_v_d = din("bvbc_v", [L, 128, D], F32)
    bvbc_e_d = din("bvbc_e", [L, 128, D], F32)
    bvrow_v_d = din("bvrow_v", [L, 1, 2 * D])
    bvrow_e_d = din("bvrow_e", [L, 1, 2 * D])
    Wupd_d = din("W_upd", [4, D, D])
    updb_d = din("updb_bc", [128, D], F32)
    Gk_v_d = din("Gk_v", [4, 128])
    Gq4_v_d = din("Gq4_v", [4, 512])
    Gk_e_d = din("Gk_e", [8, 128])
    Gq4_e_d = din("Gq4_e", [8, 512])
    ident_d = din("ident", [128, 128])
    ones1_d = din("ones1", [128, 128])

    A_d = nc.dram_tensor("A", [R, D], F32, kind="ExternalOutput").ap()
    P3_d = nc.dram_tensor("P3", [R, D], F32, kind="ExternalOutput").ap()

    with tile.TileContext(nc) as tc, ExitStack() as es, \
            nc.allow_low_precision(reason="bf16 matmul pipeline, fp32 accum in PSUM"):
        const = es.enter_context(tc.tile_pool(name="const", bufs=1))
        sb = es.enter_context(tc.tile_pool(name="sb", bufs=4))
        inp = es.enter_context(tc.tile_pool(name="inp", bufs=4))
        outp = es.enter_context(tc.tile_pool(name="outp", bufs=4))
        psmm = es.enter_context(tc.tile_pool(name="psmm", bufs=3, space="PSUM"))
        psS_pool = es.enter_context(tc.tile_pool(name="psS", bufs=2, space="PSUM"))
        psO_pool = es.enter_context(tc.tile_pool(name="psO", bufs=2, space="PSUM"))
        psT_pool = es.enter_context(tc.tile_pool(name="psT", bufs=1, space="PSUM"))
        pools = (sb, psmm, psS_pool, psO_pool, psT_pool)

        negc = const.tile([128, 1], F32, tag="negc", name="negc")
        nc.vector.memset(negc[:], -MASK_C)
        ones128 = const.tile([128, 128], BF16, tag="ones128", name="ones128")
        nc.sync.dma_start(ones128[:], ones1_d)
        ident = const.tile([128, 128], BF16, tag="ident", name="ident")
        nc.sync.dma_start(ident[:], ident_d)

        peW = {}
        for s, peW_d in (("v", peW_v_d), ("e", peW_e_d)):
            t = const.tile([WD + 1, D], BF16, tag=f"peW_{s}", name=f"peW_{s}")
            nc.sync.dma_start(t[:], peW_d)
            peW[s] = t

        side_consts = {
            "v": _load_side_consts(nc, const, "v", Wv_d, Bcol_v_d, bvbc_v_d,
                                   bvrow_v_d, Gk_v_d, Gq4_v_d, 4),
            "e": _load_side_consts(nc, const, "e", We_d, Bcol_e_d, bvbc_e_d,
                                   bvrow_e_d, Gk_e_d, Gq4_e_d, 8),
        }

        Wupd = []
        for j in range(4):
            Wupd.append([])
            for kb in range(2):
                t = const.tile([128, 256], BF16, tag=f"Wupd{j}{kb}", name=f"Wupd{j}{kb}")
                nc.sync.dma_start(t[:], Wupd_d[j, kb * 128:(kb + 1) * 128, :])
                Wupd[j].append(t)
        updb = const.tile([128, 256], F32, tag="updb", name="updb")
        nc.sync.dma_start(updb[:], updb_d)

        for side in ("v", "e"):
            Ws, Bcols, bvbcs, bvrows, Gk, Gq4 = side_consts[side]
            xt_d, wt_d = (xvt_d, wvt_d) if side == "v" else (xet_d, wet_d)
            for t in range(NT):
                cs = slice(t * 256, (t + 1) * 256)
                # xt [128, 512] bf16, col = fb*256 + r (same layout as Xt)
                xt = inp.tile([128, 512], BF16, tag=f"xt{side}", name="xt")
                for fb in range(2):
                    nc.sync.dma_start(xt[:, fb * 256:(fb + 1) * 256],
                                      xt_d[fb * 128:(fb + 1) * 128, cs])
                wt = inp.tile([WD + 1, 256], BF16, tag=f"wt{side}", name="wt")
                nc.sync.dma_start(wt[:], wt_d[:, cs])

                # mailbox: Xt = xt + peW_aug^T wt_aug (peb rides the ones row)
                Xt = sb.tile([128, 512], BF16, tag="Zt", name="Xt")
                psP = psmm.tile([128, 512], F32, tag="mm", name="psP")
                for fb in range(2):
                    nc.tensor.matmul(psP[:, fb * 256:(fb + 1) * 256],
                                     peW[side][:, fb * 128:(fb + 1) * 128],
                                     wt[:], start=True, stop=True)
                nc.vector.tensor_add(Xt[:], psP[:], xt[:])

                for l in range(L):
                    if stage <= 0:
                        break
                    Xt = _sab_tile(nc, pools, Xt, Ws[l], Bcols[l], bvbcs[l],
                                   bvrows[l], Gk, Gq4, ident, ones128, negc,
                                   v_on_act=(l == 1), stage=stage)

                if side == "v":
                    x0 = inp.tile([128, 512], BF16, tag="x0", name="x0")
                    for fb in range(2):
                        nc.sync.dma_start(x0[:, fb * 256:(fb + 1) * 256],
                                          x0t_d[fb * 128:(fb + 1) * 128, cs])
                    psA = psmm.tile([128, 512], F32, tag="mm", name="psA")
                    for rb in range(2):
                        acs = slice(rb * 256, (rb + 1) * 256)
                        first = True
                        for src, j in ((xt, 0), (Xt, 1), (x0, 3)):
                            for kb in range(2):
                                nc.tensor.matmul(
                                    psA[:, acs],
                                    src[:, kb * 256 + rb * 128:kb * 256 + (rb + 1) * 128],
                                    Wupd[j][kb][:],
                                    start=first, stop=(src is x0 and kb == 1))
                                first = False
                    Ao = outp.tile([128, 512], F32, tag="Aout", name="Aout")
                    ub = updb[:].unsqueeze(1).broadcast_to([128, 2, 256])
                    nc.vector.tensor_add(Ao[:], psA[:], ub)
                    for rb in range(2):
                        nc.sync.dma_start(
                            A_d[t * 256 + rb * 128:t * 256 + (rb + 1) * 128, :],
                            Ao[:, rb * 256:(rb + 1) * 256])
                else:
                    psP3 = psmm.tile([128, 512], F32, tag="mm", name="psP3")
                    for rb in range(2):
                        acs = slice(rb * 256, (rb + 1) * 256)
                        for kb in range(2):
                            nc.tensor.matmul(
                                psP3[:, acs],
                                Xt[:, kb * 256 + rb * 128:kb * 256 + (rb + 1) * 128],
                                Wupd[2][kb][:],
                                start=(kb == 0), stop=(kb == 1))
                    Po = outp.tile([128, 512], F32, tag="Pout", name="Pout")
                    nc.scalar.copy(Po[:], psP3[:])
                    for rb in range(2):
                        nc.sync.dma_start(
                            P3_d[t * 256 + rb * 128:t * 256 + (rb + 1) * 128, :],
                            Po[:, rb * 256:(rb + 1) * 256])

    nc.compile()
    return nc


def _make_group_consts(n_group):
    """Gk [G,128] (16C in-group), Gq4 [G,512] (q one-hot, tiled 4x for heads)."""
    G = 128 // n_group
    Gk = np.zeros((G, 128), np.float32)
    Gq = np.zeros((G, 128), np.float32)
    for g in range(G):
        Gk[g, g * n_group:(g + 1) * n_group] = 16.0 * MASK_C
        Gq[g, g * n_group:(g + 1) * n_group] = 1.0
    Gq4 = np.tile(Gq, (1, 4))
    return Gk.astype(NPBF16), Gq4.astype(NPBF16)


def _pack_bcol(bq, bk, bv, bo):
    """[128, 8] bias columns per layer: col p*2+fb."""
    out = np.zeros((L, 128, 8), np.float32)
    for l in range(L):
        for pi, b in enumerate((bq, bk, bv, bo)):
            for fb in range(2):
                out[l, :, pi * 2 + fb] = b[l, fb * 128:(fb + 1) * 128]
    return out


_PROGRAM_CACHE = {}


def _get_program(R):
    if R not in _PROGRAM_CACHE:
        _PROGRAM_CACHE[R] = build_program(R)
    return _PROGRAM_CACHE[R]


def kernel(co_feat_in, co_feat_con, co_feat_0, weight_in, weight_con,
           pe_v_W, pe_v_b, pe_e_W, pe_e_b,
           Wq_v, bq_v, Wk_v, bk_v, Wv_v, bv_v, Wo_v, bo_v,
           Wq_e, bq_e, Wk_e, bk_e, Wv_e, bv_e, Wo_e, bo_e,
           upd_W, upd_b, perm):
    f = np.asarray
    perm = np.asarray(perm)

    R = E // NCORES
    nc = _get_program(R)

    Gk_v, Gq4_v = _make_group_consts(DV)
    Gk_e, Gq4_e = _make_group_consts(DE)

    def bfT(a):
        """[rows, feat] fp32 -> contiguous [feat, rows] bf16."""
        return np.ascontiguousarray(f(a, np.float32).T).astype(NPBF16)

    def aug_ones(a):
        """[WD, R] -> [WD+1, R] with a trailing ones row."""
        return np.concatenate([a, np.ones((1, a.shape[1]), a.dtype)], axis=0)

    def aug_bias(w, b):
        """[WD, D] + [D] -> [WD+1, D] with the bias as the trailing row."""
        return np.concatenate(
            [f(w, np.float32), f(b, np.float32).reshape(1, -1)], axis=0)

    xvt = bfT(co_feat_in)
    wvt = aug_ones(bfT(weight_in))
    xet = bfT(co_feat_con)
    wet = aug_ones(bfT(weight_con))
    x0t = bfT(co_feat_0)

    shared = {
        "peW_v": aug_bias(pe_v_W, pe_v_b).astype(NPBF16),
        "peW_e": aug_bias(pe_e_W, pe_e_b).astype(NPBF16),
        "W_v": np.stack([f(Wq_v, np.float32), f(Wk_v, np.float32),
                         f(Wv_v, np.float32), f(Wo_v, np.float32)],
                        axis=1).astype(NPBF16),
        "W_e": np.stack([f(Wq_e, np.float32), f(Wk_e, np.float32),
                         f(Wv_e, np.float32), f(Wo_e, np.float32)],
                        axis=1).astype(NPBF16),
        "Bcol_v": _pack_bcol(f(bq_v, np.float32), f(bk_v, np.float32),
                             f(bv_v, np.float32), f(bo_v, np.float32)),
        "Bcol_e": _pack_bcol(f(bq_e, np.float32), f(bk_e, np.float32),
                             f(bv_e, np.float32), f(bo_e, np.float32)),
        "bvbc_v": np.ascontiguousarray(
            np.broadcast_to(f(bv_v, np.float32)[:, None, :], (L, 128, D))),
        "bvbc_e": np.ascontiguousarray(
            np.broadcast_to(f(bv_e, np.float32)[:, None, :], (L, 128, D))),
        "bvrow_v": np.tile(f(bv_v, np.float32).reshape(L, 1, D),
                           (1, 1, 2)).astype(NPBF16),
        "bvrow_e": np.tile(f(bv_e, np.float32).reshape(L, 1, D),
                           (1, 1, 2)).astype(NPBF16),
        "W_upd": f(upd_W, np.float32).reshape(4, D, D).astype(NPBF16),
        "updb_bc": np.ascontiguousarray(
            np.broadcast_to(f(upd_b, np.float32)[None, :], (128, D))),
        "Gk_v": Gk_v, "Gq4_v": Gq4_v, "Gk_e": Gk_e, "Gq4_e": Gq4_e,
        "ident": np.eye(128, dtype=np.float32).astype(NPBF16),
        "ones1": np.ones((128, 128), NPBF16),
    }

    in_maps = []
    for c in range(NCORES):
        rs = slice(c * R, (c + 1) * R)
        m = dict(shared)
        m["xvt"] = np.ascontiguousarray(xvt[:, rs])
        m["wvt"] = np.ascontiguousarray(wvt[:, rs])
        m["xet"] = np.ascontiguousarray(xet[:, rs])
        m["wet"] = np.ascontiguousarray(wet[:, rs])
        m["x0t"] = np.ascontiguousarray(x0t[:, rs])
        in_maps.append(m)

    res = run_bass_kernel_spmd(nc, in_maps, core_ids=list(range(NCORES)))
    A = np.concatenate([res.results[c]["A"] for c in range(NCORES)], axis=0)
    P3 = np.concatenate([res.results[c]["P3"] for c in range(NCORES)], axis=0)

    inv_perm = np.argsort(perm)
    out_in = A + P3[inv_perm]
    return np.stack([out_in, out_in[perm]]).astype(np.float32)
